# revision 10
# baseline (speedup 1.0000x reference)
"""Additive (Bahdanau) attention on 8 TRN2 NeuronCores — V2.1.

Reference computation:
    qp = queries @ W_q                  (bs, n_q, 64)
    kp = keys @ W_k                     (bs, n_k, 64)
    scores[b,q,k] = sum_h w_v[h] * tanh(qp[b,q,h] + kp[b,k,h])
    out = softmax(scores, -1) @ values

tanh(x) on [-9.2, 9.2] ~= sum_j c_j sin((2j+1) w0 x), J=5 (refit, max err
1.7e-2, e2e ~1.25e-2 vs the 2e-2 gate); angle addition makes the scores
separable into matmuls with contraction 2*64 per harmonic. Range
reduction for j >= 1 via fp32 bit surgery (z in [32,64) -> frac is the
low 18 mantissa bits), then sin(2 pi z) = Sin(-64 pi v + 65 pi).

V2.1 vs V1:
  - k-side DMA + transposes + projection first (they gate the Scalar Sin
    chain); projections in float32r (1 cyc/row vs fp32's 4).
  - j=0 Sin reads the projection PSUM directly; SBUF copies overlap.
  - harmonic pair (2,3) as fp8e4 DoubleRow matmuls (2 harmonics per pass).
  - one merged bit-surgery tensor_scalar per j (k and q columns adjacent).
  - q-side scale+cast on DVE (j=0 on ScalarE); GpSimd only issues DMAs
    (its elementwise ops measure ~10us per [128,512] tile and its SBUF
    traffic starves the DVE through the shared ports).
  - Exp's activation-table load hides behind the j=4 score matmuls.
  - merged output DMA (one descriptor).

Sharding: fully data-parallel, no collectives. Core c handles batch c//2,
query half c%2: (512 q, 1024 k).
"""

import numpy as np

BS, NQ, NK = 4, 1024, 1024
QD, KD, VD, HID = 128, 128, 128, 64
NCORES = 8
NQH = NQ // 2  # queries per core

J = 5
W0 = 0.263343
FOURIER_C = [1.238084, 0.332728, 0.135246, 0.058602, 0.02743]

TWO_PI = 6.283185307179586
HALF_PI = 1.5707963267948966
PI64 = 64 * 3.141592653589793

_CACHED = {}


def _build():
    import concourse.bacc as bacc
    import concourse.mybir as mybir
    from concourse import tile
    from concourse.alu_op_type import AluOpType
    from concourse.masks import make_identity

    F32 = mybir.dt.float32
    F32R = mybir.dt.float32r
    U32 = mybir.dt.uint32
    BF16 = mybir.dt.bfloat16
    FP8 = mybir.dt.float8e4
    A = mybir.ActivationFunctionType
    DR = mybir.MatmulPerfMode.DoubleRow

    nc = bacc.Bacc(None, target_bir_lowering=False)

    q_sh = nc.declare_dram_parameter("q_sh", [NQH, QD], F32, isOutput=False)
    k_sh = nc.declare_dram_parameter("k_sh", [NK, KD], F32, isOutput=False)
    v_sh = nc.declare_dram_parameter("v_sh", [NK, VD], F32, isOutput=False)
    wqk = nc.declare_dram_parameter("wqk", [128, 256], F32R, isOutput=False)
    cvec = nc.declare_dram_parameter("cvec", [128, 16], F32, isOutput=False)
    out = nc.declare_dram_parameter("out", [NQH, VD], F32, isOutput=True)

    NQC = NQH // 128  # 4 query chunks
    NKC = NK // 128   # 8 key chunks

    with tile.TileContext(nc) as tc:
        with (
            tc.tile_pool(name="consts", bufs=1) as consts,
            tc.tile_pool(name="io", bufs=1) as io,
            tc.tile_pool(name="work", bufs=2) as work,
            tc.tile_pool(name="jb", bufs=3) as jb,
            tc.tile_pool(name="sm", bufs=NKC) as sm,
            tc.tile_pool(name="ps", bufs=8, space="PSUM") as ps,
        ):
            # ---- input DMAs first: k gates the Sin chain. The warm Sin
            # (which triggers the table loads) must come AFTER the scalar
            # queue's dma_starts — table loads block the DGE queue.
            kstage = []
            for h in range(2):
                kst = io.tile([128, 4, 128], F32, tag=f"kst{h}")
                eng = nc.scalar if h == 0 else nc.sync
                eng.dma_start(
                    kst[:],
                    k_sh[h * 512:(h + 1) * 512, :].rearrange(
                        "(c p) d -> p c d", p=128))
                kstage.append(kst)
            qstage = []
            for h in range(2):
                qst = io.tile([128, 2, 128], F32, tag=f"qst{h}")
                eng = nc.scalar if h == 0 else nc.sync
                eng.dma_start(
                    qst[:],
                    q_sh[h * 256:(h + 1) * 256, :].rearrange(
                        "(c p) d -> p c d", p=128))
                qstage.append(qst)

            ones16 = consts.tile([128, 1], BF16, tag="ones16")
            nc.gpsimd.memset(ones16[:], 1.0)
            warm = consts.tile([1, 1], F32, tag="warm")
            nc.scalar.activation(warm[:], ones16[:1, :1], A.Sin)
            id32 = consts.tile([128, 128], F32, tag="id32")
            make_identity(nc, id32[:])

            cvec_sb = consts.tile([128, 16], F32, tag="cvec")
            wqk_sb = consts.tile([128, 256], F32R, tag="wqk")
            nc.gpsimd.dma_start(cvec_sb[:], cvec[:, :])
            nc.gpsimd.dma_start(wqk_sb[:], wqk[:, :])
            sphq = cvec_sb[:, 0:1]
            sphk = cvec_sb[:, 1:2]
            biasq = cvec_sb[:, 2:3]
            biask = cvec_sb[:, 3:4]
            bias65 = cvec_sb[:, 4:5]

            id16 = consts.tile([128, 128], BF16, tag="id16")
            make_identity(nc, id16[:])
            # values: needed only at the tail; issue after consts on gpsimd
            vstage = []
            for h in range(2):
                vst = io.tile([128, 4, 128], F32, tag=f"vst{h}")
                nc.gpsimd.dma_start(
                    vst[:],
                    v_sh[h * 512:(h + 1) * 512, :].rearrange(
                        "(c p) d -> p c d", p=128))
                vstage.append(vst)

            # ---- k: transpose + f32r projection per half; Sin j=0 reads
            # the PSUM directly while the SBUF copies drain.
            wq_r = wqk_sb[:, 0:128]
            wk_r = wqk_sb[:, 128:256]
            kT = io.tile([KD, NK], F32R, tag="kT")
            kp2 = io.tile([128, NK], F32, tag="kp2")
            ps_kp = []
            for h in range(2):
                pk = ps.tile([128, 512], F32, tag="t512", name=f"p_k_{h}")
                for c in range(4):
                    nc.tensor.transpose(pk[:, c * 128:(c + 1) * 128],
                                        kstage[h][:, c, :], id32[:])
                nc.vector.tensor_copy(kT[:, h * 512:(h + 1) * 512], pk[:])
                pk2 = ps.tile([128, 512], F32, tag="t512", name=f"ps_kp_{h}")
                nc.tensor.matmul(
                    pk2[:], wk_r, kT[:, h * 512:(h + 1) * 512],
                    start=True, stop=True)
                nc.vector.tensor_copy(kp2[:, h * 512:(h + 1) * 512], pk2[:])
                ps_kp.append(pk2)

            qT = io.tile([QD, NQH], F32R, tag="qT")
            qp2 = io.tile([128, NQH], F32, tag="qp2")
            p_q = ps.tile([128, 512], F32, tag="t512", name="p_q")
            for h in range(2):
                for c in range(2):
                    i = h * 2 + c
                    nc.tensor.transpose(p_q[:, i * 128:(i + 1) * 128],
                                        qstage[h][:, c, :], id32[:])
            nc.vector.tensor_copy(qT[:], p_q[:])
            ps_qp = ps.tile([128, 512], F32, tag="t512", name="ps_qp")
            nc.tensor.matmul(ps_qp[:], wq_r, qT[:], start=True, stop=True)
            nc.vector.tensor_copy(qp2[:], ps_qp[:])

            # ---- per-j trig banks ----
            # K rows [cos | sin] unscaled; Q rows [sin | cos] * c_j*w_v.
            # j=0,1,4: bf16; (2,3): fp8e4 packed in a DoubleRow pair tile.
            ksb = {j: jb.tile([128, NK], BF16, tag="ks", name=f"ks{j}")
                   for j in (0, 1, 4)}
            sqb = {j: jb.tile([128, NQH], BF16, tag="sq", name=f"sq{j}")
                   for j in (0, 1, 4)}
            kspair = jb.tile([128, 2, NK], FP8, tag="kspair")
            sqpair = jb.tile([128, 2, NQH], FP8, tag="sqpair")

            psT = [ps.tile([128, 512], F32, tag="t512", name=f"psT_{kt}")
                   for kt in range(NKC)]

            for j in range(J):
                if j in (2, 3):
                    ks_dst = kspair[:, j - 2, :]
                    sq_dst = sqpair[:, j - 2, :]
                else:
                    ks_dst = ksb[j][:]
                    sq_dst = sqb[j][:]
                sqf = work.tile([128, NQH], F32, tag="sqf", name=f"sqf{j}")
                if j == 0:
                    # |w0 x| + pi/2 < 2.9: direct Sin, straight from PSUM
                    nc.scalar.activation(ks_dst[:, 0:512], ps_kp[0][:],
                                         A.Sin, bias=biask, scale=W0)
                    nc.scalar.activation(ks_dst[:, 512:1024], ps_kp[1][:],
                                         A.Sin, bias=biask, scale=W0)
                    nc.scalar.activation(sqf[:], ps_qp[:],
                                         A.Sin, bias=biasq, scale=W0)
                else:
                    s1 = float((2 * j + 1) * W0 / TWO_PI)
                    zqk = work.tile([128, NK + NQH], F32, tag="zqk",
                                    name=f"zqk{j}")
                    vqk = work.tile([128, NK + NQH], F32, tag="vqk",
                                    name=f"vqk{j}")
                    nc.vector.tensor_scalar(zqk[:, 0:NK], kp2[:], s1, sphk,
                                            AluOpType.mult, AluOpType.add)
                    nc.vector.tensor_scalar(zqk[:, NK:], qp2[:], s1, sphq,
                                            AluOpType.mult, AluOpType.add)
                    nc.vector.tensor_scalar(vqk[:].bitcast(U32),
                                            zqk[:].bitcast(U32),
                                            0x0003FFFF, 0x3F800000,
                                            AluOpType.bitwise_and,
                                            AluOpType.bitwise_or)
                    nc.scalar.activation(ks_dst, vqk[:, 0:NK], A.Sin,
                                         scale=-PI64, bias=bias65)
                    nc.scalar.activation(sqf[:], vqk[:, NK:], A.Sin,
                                         scale=-PI64, bias=bias65)
                # c_j*w_v scale + cast: ScalarE for j=0 (right after its own
                # Sin; DVE still busy with head copies), DVE for the rest
                cw_j = cvec_sb[:, 5 + j:6 + j]
                if j == 0:
                    nc.scalar.mul(sq_dst, sqf[:], cw_j)
                else:
                    nc.vector.tensor_scalar_mul(sq_dst, sqf[:], cw_j)

                if j in (0, 1, 4):
                    for kt in range(NKC):
                        nc.tensor.matmul(
                            psT[kt][:], ksb[j][:, kt * 128:(kt + 1) * 128],
                            sqb[j][:], start=(j == 0), stop=(j == 4))
                elif j == 3:
                    for kt in range(NKC):
                        nc.tensor.matmul(
                            psT[kt][:],
                            kspair[:, :, kt * 128:(kt + 1) * 128],
                            sqpair[:], start=False, stop=False,
                            perf_mode=DR)

            # ---- exp (k-major) + denominators + output matmuls ----
            v16 = []
            for h in range(2):
                vb = sm.tile([128, 4, 128], BF16, tag=f"v16_{h}")
                nc.vector.tensor_copy(vb[:], vstage[h][:])
                v16.append(vb)
            expT = []
            for kt in range(NKC):
                et = sm.tile([128, 512], BF16, tag="expT", name=f"expT_{kt}")
                nc.scalar.activation(et[:], psT[kt][:], A.Exp)
                expT.append(et)
            psum_sums = ps.tile([1, 512], F32, tag="t512", name="psum_sums")
            for kt in range(NKC):
                nc.tensor.matmul(psum_sums[:], ones16[:], expT[kt][:],
                                 start=(kt == 0), stop=(kt == NKC - 1))
            sums_sb = sm.tile([1, 512], F32, tag="sums_sb")
            nc.scalar.copy(sums_sb[:], psum_sums[:])

            ps_outT = ps.tile([128, 512], F32, tag="t512", name="ps_outT")
            for kt in range(NKC):
                nc.tensor.matmul(ps_outT[:], v16[kt // 4][:, kt % 4, :],
                                 expT[kt][:], start=(kt == 0),
                                 stop=(kt == NKC - 1))
            outT_sb = sm.tile([128, 512], BF16, tag="outT_sb")
            nc.vector.tensor_copy(outT_sb[:], ps_outT[:])

            # ---- transpose back to (q, v), normalize, store in 2 halves --
            o_all = sm.tile([128, NQC, 128], F32, tag="o_all")
            pcol = ps.tile([128, 512], F32, tag="t512", name="pcol")
            for qt in range(NQC):
                nc.tensor.matmul(pcol[:128, qt:qt + 1],
                                 sums_sb[:1, qt * 128:(qt + 1) * 128],
                                 id32[:1, :1], start=True, stop=True)
            rcol = sm.tile([128, NQC], F32, tag="rcol")
            nc.vector.reciprocal(rcol[:], pcol[:128, :NQC])
            po = ps.tile([128, 512], BF16, tag="t512", name="po")
            for qt in range(NQC):
                nc.tensor.transpose(po[:, qt * 128:(qt + 1) * 128],
                                    outT_sb[:, qt * 128:(qt + 1) * 128],
                                    id16[:])
            for qt in range(NQC):
                nc.vector.tensor_scalar_mul(o_all[:, qt, :],
                                            po[:, qt * 128:(qt + 1) * 128],
                                            rcol[:, qt:qt + 1])
                if qt == 1:
                    nc.sync.dma_start(
                        out[0:256, :].rearrange("(c p) d -> p c d", p=128),
                        o_all[:, 0:2, :])
            nc.sync.dma_start(
                out[256:512, :].rearrange("(c p) d -> p c d", p=128),
                o_all[:, 2:4, :])

    nc.finalize()
    return nc


def _get_nc():
    if "nc" not in _CACHED:
        _CACHED["nc"] = _build()
    return _CACHED["nc"]


def _make_consts(W_q, W_k, w_v):
    # wqk layout: [:, 0:128] = [W_q | W_q], [:, 128:256] = [W_k | W_k]
    wqk = np.zeros((128, 256), np.float32)
    wqk[:, 0:64] = W_q
    wqk[:, 64:128] = W_q
    wqk[:, 128:192] = W_k
    wqk[:, 192:256] = W_k
    cvec = np.zeros((128, 16), np.float32)
    # wrap-phase consts (turns, +36 so z lands in [32, 64)):
    # Q packing [sin | cos], K packing [cos | sin]
    cvec[:64, 0] = 36.0
    cvec[64:, 0] = 36.25   # sphq
    cvec[:64, 1] = 36.25
    cvec[64:, 1] = 36.0    # sphk
    cvec[64:, 2] = HALF_PI  # biasq (radians, j=0 direct)
    cvec[:64, 3] = HALF_PI  # biask
    cvec[:, 4] = 65 * np.pi  # bias65
    for j in range(J):
        cwj = (FOURIER_C[j] * w_v).astype(np.float32)
        cvec[:64, 5 + j] = cwj
        cvec[64:, 5 + j] = cwj
    return wqk, cvec


def kernel(queries, keys, values, W_q, W_k, w_v, _trace=False, _trace_kwargs=None):
    from concourse.bass_utils import run_bass_kernel_spmd

    nc = _get_nc()
    wqk, cvec = _make_consts(
        np.asarray(W_q), np.asarray(W_k), np.asarray(w_v))
    queries = np.ascontiguousarray(queries, np.float32)
    keys = np.ascontiguousarray(keys, np.float32)
    values = np.ascontiguousarray(values, np.float32)

    in_maps = []
    for c in range(NCORES):
        b, qh = c // 2, c % 2
        in_maps.append({
            "q_sh": np.ascontiguousarray(queries[b, qh * NQH:(qh + 1) * NQH, :]),
            "k_sh": keys[b],
            "v_sh": values[b],
            "wqk": wqk, "cvec": cvec,
        })

    kwargs = {}
    if _trace:
        kwargs["trace"] = True
        kwargs.update(_trace_kwargs or {})
    res = run_bass_kernel_spmd(nc, in_maps, core_ids=list(range(NCORES)), **kwargs)

    out = np.empty((BS, NQ, VD), np.float32)
    for c in range(NCORES):
        b, qh = c // 2, c % 2
        out[b, qh * NQH:(qh + 1) * NQH, :] = res.results[c]["out"]
    if _trace:
        return out, res
    return out


# revision 13
# speedup vs baseline: 1.0088x; 1.0088x over previous
"""Additive (Bahdanau) attention on 8 TRN2 NeuronCores — V2.1.

Reference computation:
    qp = queries @ W_q                  (bs, n_q, 64)
    kp = keys @ W_k                     (bs, n_k, 64)
    scores[b,q,k] = sum_h w_v[h] * tanh(qp[b,q,h] + kp[b,k,h])
    out = softmax(scores, -1) @ values

tanh(x) on [-9.2, 9.2] ~= sum_j c_j sin((2j+1) w0 x), J=5 (refit, max err
1.7e-2, e2e ~1.25e-2 vs the 2e-2 gate); angle addition makes the scores
separable into matmuls with contraction 2*64 per harmonic. Range
reduction for j >= 1 via fp32 bit surgery (z in [32,64) -> frac is the
low 18 mantissa bits), then sin(2 pi z) = Sin(-64 pi v + 65 pi).

V2.1 vs V1:
  - k-side DMA + transposes + projection first (they gate the Scalar Sin
    chain); projections in float32r (1 cyc/row vs fp32's 4).
  - j=0 Sin reads the projection PSUM directly; SBUF copies overlap.
  - harmonic pair (2,3) as fp8e4 DoubleRow matmuls (2 harmonics per pass).
  - one merged bit-surgery tensor_scalar per j (k and q columns adjacent).
  - q-side scale+cast on DVE (j=0 on ScalarE); GpSimd only issues DMAs
    (its elementwise ops measure ~10us per [128,512] tile and its SBUF
    traffic starves the DVE through the shared ports).
  - Exp's activation-table load hides behind the j=4 score matmuls.
  - merged output DMA (one descriptor).

Sharding: fully data-parallel, no collectives. Core c handles batch c//2,
query half c%2: (512 q, 1024 k).
"""

import numpy as np

BS, NQ, NK = 4, 1024, 1024
QD, KD, VD, HID = 128, 128, 128, 64
NCORES = 8
NQH = NQ // 2  # queries per core

J = 5
W0 = 0.263343
FOURIER_C = [1.238084, 0.332728, 0.135246, 0.058602, 0.02743]

TWO_PI = 6.283185307179586
HALF_PI = 1.5707963267948966
PI64 = 64 * 3.141592653589793

_CACHED = {}


def _build():
    import concourse.bacc as bacc
    import concourse.mybir as mybir
    from concourse import tile
    from concourse.alu_op_type import AluOpType
    from concourse.masks import make_identity

    F32 = mybir.dt.float32
    F32R = mybir.dt.float32r
    U32 = mybir.dt.uint32
    BF16 = mybir.dt.bfloat16
    FP8 = mybir.dt.float8e4
    A = mybir.ActivationFunctionType
    DR = mybir.MatmulPerfMode.DoubleRow

    nc = bacc.Bacc(None, target_bir_lowering=False)

    q_sh = nc.declare_dram_parameter("q_sh", [NQH, QD], F32, isOutput=False)
    k_sh = nc.declare_dram_parameter("k_sh", [NK, KD], F32, isOutput=False)
    v_sh = nc.declare_dram_parameter("v_sh", [NK, VD], F32, isOutput=False)
    wqk = nc.declare_dram_parameter("wqk", [128, 256], F32R, isOutput=False)
    cvec = nc.declare_dram_parameter("cvec", [128, 16], F32, isOutput=False)
    out = nc.declare_dram_parameter("out", [NQH, VD], F32, isOutput=True)

    NQC = NQH // 128  # 4 query chunks
    NKC = NK // 128   # 8 key chunks

    with tile.TileContext(nc) as tc:
        with (
            tc.tile_pool(name="consts", bufs=1) as consts,
            tc.tile_pool(name="io", bufs=1) as io,
            tc.tile_pool(name="work", bufs=2) as work,
            tc.tile_pool(name="jb", bufs=3) as jb,
            tc.tile_pool(name="sm", bufs=NKC) as sm,
            tc.tile_pool(name="ps", bufs=8, space="PSUM") as ps,
        ):
            # ---- input DMAs first: k gates the Sin chain. The warm Sin
            # (which triggers the table loads) must come AFTER the scalar
            # queue's dma_starts — table loads block the DGE queue.
            # k/v row order is free (softmax + output contract over k), so
            # load them fully contiguous: partition p holds rows 8p..8p+7.
            # q rows are permuted the same way (4p..4p+3); the host gathers
            # the output rows back (see kernel()).
            kstage = []
            for h in range(2):
                kst = io.tile([128, 4, 128], F32, tag=f"kst{h}")
                eng = nc.scalar if h == 0 else nc.sync
                eng.dma_start(
                    kst[:],
                    k_sh[:, :].rearrange("(p c) d -> p c d", p=128)[
                        :, h * 4:(h + 1) * 4, :])
                kstage.append(kst)
            qstage = []
            for h in range(2):
                qst = io.tile([128, 2, 128], F32, tag=f"qst{h}")
                eng = nc.scalar if h == 0 else nc.sync
                eng.dma_start(
                    qst[:],
                    q_sh[:, :].rearrange("(p c) d -> p c d", p=128)[
                        :, h * 2:(h + 1) * 2, :])
                qstage.append(qst)

            ones16 = consts.tile([128, 1], BF16, tag="ones16")
            nc.gpsimd.memset(ones16[:], 1.0)
            warm = consts.tile([1, 1], F32, tag="warm")
            nc.scalar.activation(warm[:], ones16[:1, :1], A.Sin)
            id32 = consts.tile([128, 128], F32, tag="id32")
            make_identity(nc, id32[:])

            cvec_sb = consts.tile([128, 16], F32, tag="cvec")
            wqk_sb = consts.tile([128, 256], F32R, tag="wqk")
            nc.gpsimd.dma_start(cvec_sb[:], cvec[:, :])
            nc.gpsimd.dma_start(wqk_sb[:], wqk[:, :])
            sphq = cvec_sb[:, 0:1]
            sphk = cvec_sb[:, 1:2]
            biasq = cvec_sb[:, 2:3]
            biask = cvec_sb[:, 3:4]
            bias65 = cvec_sb[:, 4:5]

            id16 = consts.tile([128, 128], BF16, tag="id16")
            make_identity(nc, id16[:])
            # values: needed only at the tail; issue after consts on gpsimd
            vstage = []
            for h in range(2):
                vst = io.tile([128, 4, 128], F32, tag=f"vst{h}")
                nc.gpsimd.dma_start(
                    vst[:],
                    v_sh[:, :].rearrange("(p c) d -> p c d", p=128)[
                        :, h * 4:(h + 1) * 4, :])
                vstage.append(vst)

            # ---- k: transpose + f32r projection per half; Sin j=0 reads
            # the PSUM directly while the SBUF copies drain.
            wq_r = wqk_sb[:, 0:128]
            wk_r = wqk_sb[:, 128:256]
            kT = io.tile([KD, NK], F32R, tag="kT")
            kp2 = io.tile([128, NK], F32, tag="kp2")
            ps_kp = []
            for h in range(2):
                pk = ps.tile([128, 512], F32, tag="t512", name=f"p_k_{h}")
                for c in range(4):
                    nc.tensor.transpose(pk[:, c * 128:(c + 1) * 128],
                                        kstage[h][:, c, :], id32[:])
                nc.vector.tensor_copy(kT[:, h * 512:(h + 1) * 512], pk[:])
                pk2 = ps.tile([128, 512], F32, tag="t512", name=f"ps_kp_{h}")
                nc.tensor.matmul(
                    pk2[:], wk_r, kT[:, h * 512:(h + 1) * 512],
                    start=True, stop=True)
                nc.vector.tensor_copy(kp2[:, h * 512:(h + 1) * 512], pk2[:])
                ps_kp.append(pk2)

            qT = io.tile([QD, NQH], F32R, tag="qT")
            qp2 = io.tile([128, NQH], F32, tag="qp2")
            p_q = ps.tile([128, 512], F32, tag="t512", name="p_q")
            for h in range(2):
                for c in range(2):
                    i = h * 2 + c
                    nc.tensor.transpose(p_q[:, i * 128:(i + 1) * 128],
                                        qstage[h][:, c, :], id32[:])
            nc.vector.tensor_copy(qT[:], p_q[:])
            ps_qp = ps.tile([128, 512], F32, tag="t512", name="ps_qp")
            nc.tensor.matmul(ps_qp[:], wq_r, qT[:], start=True, stop=True)
            nc.vector.tensor_copy(qp2[:], ps_qp[:])

            # ---- per-j trig banks ----
            # K rows [cos | sin] unscaled; Q rows [sin | cos] * c_j*w_v.
            # j=0,1,4: bf16; (2,3): fp8e4 packed in a DoubleRow pair tile.
            ksb = {j: jb.tile([128, NK], BF16, tag="ks", name=f"ks{j}")
                   for j in (0, 1, 4)}
            sqb = {j: jb.tile([128, NQH], BF16, tag="sq", name=f"sq{j}")
                   for j in (0, 1, 4)}
            kspair = jb.tile([128, 2, NK], FP8, tag="kspair")
            sqpair = jb.tile([128, 2, NQH], FP8, tag="sqpair")

            psT = [ps.tile([128, 512], F32, tag="t512", name=f"psT_{kt}")
                   for kt in range(NKC)]

            for j in range(J):
                if j in (2, 3):
                    ks_dst = kspair[:, j - 2, :]
                    sq_dst = sqpair[:, j - 2, :]
                else:
                    ks_dst = ksb[j][:]
                    sq_dst = sqb[j][:]
                sqf = work.tile([128, NQH], F32, tag="sqf", name=f"sqf{j}")
                if j == 0:
                    # |w0 x| + pi/2 < 2.9: direct Sin, straight from PSUM
                    nc.scalar.activation(ks_dst[:, 0:512], ps_kp[0][:],
                                         A.Sin, bias=biask, scale=W0)
                    nc.scalar.activation(ks_dst[:, 512:1024], ps_kp[1][:],
                                         A.Sin, bias=biask, scale=W0)
                    nc.scalar.activation(sqf[:], ps_qp[:],
                                         A.Sin, bias=biasq, scale=W0)
                else:
                    s1 = float((2 * j + 1) * W0 / TWO_PI)
                    zqk = work.tile([128, NK + NQH], F32, tag="zqk",
                                    name=f"zqk{j}")
                    vqk = work.tile([128, NK + NQH], F32, tag="vqk",
                                    name=f"vqk{j}")
                    nc.vector.tensor_scalar(zqk[:, 0:NK], kp2[:], s1, sphk,
                                            AluOpType.mult, AluOpType.add)
                    nc.vector.tensor_scalar(zqk[:, NK:], qp2[:], s1, sphq,
                                            AluOpType.mult, AluOpType.add)
                    nc.vector.tensor_scalar(vqk[:].bitcast(U32),
                                            zqk[:].bitcast(U32),
                                            0x0003FFFF, 0x3F800000,
                                            AluOpType.bitwise_and,
                                            AluOpType.bitwise_or)
                    nc.scalar.activation(ks_dst, vqk[:, 0:NK], A.Sin,
                                         scale=-PI64, bias=bias65)
                    nc.scalar.activation(sqf[:], vqk[:, NK:], A.Sin,
                                         scale=-PI64, bias=bias65)
                # c_j*w_v scale + cast: ScalarE for j=0 (right after its own
                # Sin; DVE still busy with head copies), DVE for the rest
                cw_j = cvec_sb[:, 5 + j:6 + j]
                if j == 0:
                    nc.scalar.mul(sq_dst, sqf[:], cw_j)
                else:
                    nc.vector.tensor_scalar_mul(sq_dst, sqf[:], cw_j)

                if j in (0, 1, 4):
                    for kt in range(NKC):
                        nc.tensor.matmul(
                            psT[kt][:], ksb[j][:, kt * 128:(kt + 1) * 128],
                            sqb[j][:], start=(j == 0), stop=(j == 4))
                elif j == 3:
                    for kt in range(NKC):
                        nc.tensor.matmul(
                            psT[kt][:],
                            kspair[:, :, kt * 128:(kt + 1) * 128],
                            sqpair[:], start=False, stop=False,
                            perf_mode=DR)

            # ---- exp (k-major) + denominators + output matmuls ----
            v16 = []
            for h in range(2):
                vb = sm.tile([128, 4, 128], BF16, tag=f"v16_{h}")
                nc.vector.tensor_copy(vb[:], vstage[h][:])
                v16.append(vb)
            expT = []
            for kt in range(NKC):
                et = sm.tile([128, 512], BF16, tag="expT", name=f"expT_{kt}")
                nc.scalar.activation(et[:], psT[kt][:], A.Exp)
                expT.append(et)
            psum_sums = ps.tile([1, 512], F32, tag="t512", name="psum_sums")
            for kt in range(NKC):
                nc.tensor.matmul(psum_sums[:], ones16[:], expT[kt][:],
                                 start=(kt == 0), stop=(kt == NKC - 1))
            sums_sb = sm.tile([1, 512], F32, tag="sums_sb")
            nc.scalar.copy(sums_sb[:], psum_sums[:])

            ps_outT = ps.tile([128, 512], F32, tag="t512", name="ps_outT")
            for kt in range(NKC):
                nc.tensor.matmul(ps_outT[:], v16[kt // 4][:, kt % 4, :],
                                 expT[kt][:], start=(kt == 0),
                                 stop=(kt == NKC - 1))
            outT_sb = sm.tile([128, 512], BF16, tag="outT_sb")
            nc.vector.tensor_copy(outT_sb[:], ps_outT[:])

            # ---- transpose back to (q, v), normalize, store in 2 halves --
            o_all = sm.tile([128, NQC, 128], F32, tag="o_all")
            pcol = ps.tile([128, 512], F32, tag="t512", name="pcol")
            for qt in range(NQC):
                nc.tensor.matmul(pcol[:128, qt:qt + 1],
                                 sums_sb[:1, qt * 128:(qt + 1) * 128],
                                 id32[:1, :1], start=True, stop=True)
            rcol = sm.tile([128, NQC], F32, tag="rcol")
            nc.vector.reciprocal(rcol[:], pcol[:128, :NQC])
            po = ps.tile([128, 512], BF16, tag="t512", name="po")
            for qt in range(NQC):
                nc.tensor.transpose(po[:, qt * 128:(qt + 1) * 128],
                                    outT_sb[:, qt * 128:(qt + 1) * 128],
                                    id16[:])
            for qt in range(NQC):
                nc.vector.tensor_scalar_mul(o_all[:, qt, :],
                                            po[:, qt * 128:(qt + 1) * 128],
                                            rcol[:, qt:qt + 1])
                if qt == 1:
                    nc.sync.dma_start(
                        out[:, :].rearrange("(p c) d -> p c d", p=128)[
                            :, 0:2, :],
                        o_all[:, 0:2, :])
            nc.sync.dma_start(
                out[:, :].rearrange("(p c) d -> p c d", p=128)[:, 2:4, :],
                o_all[:, 2:4, :])

    nc.finalize()
    return nc


def _get_nc():
    if "nc" not in _CACHED:
        _CACHED["nc"] = _build()
    return _CACHED["nc"]


def _make_consts(W_q, W_k, w_v):
    # wqk layout: [:, 0:128] = [W_q | W_q], [:, 128:256] = [W_k | W_k]
    wqk = np.zeros((128, 256), np.float32)
    wqk[:, 0:64] = W_q
    wqk[:, 64:128] = W_q
    wqk[:, 128:192] = W_k
    wqk[:, 192:256] = W_k
    cvec = np.zeros((128, 16), np.float32)
    # wrap-phase consts (turns, +36 so z lands in [32, 64)):
    # Q packing [sin | cos], K packing [cos | sin]
    cvec[:64, 0] = 36.0
    cvec[64:, 0] = 36.25   # sphq
    cvec[:64, 1] = 36.25
    cvec[64:, 1] = 36.0    # sphk
    cvec[64:, 2] = HALF_PI  # biasq (radians, j=0 direct)
    cvec[:64, 3] = HALF_PI  # biask
    cvec[:, 4] = 65 * np.pi  # bias65
    for j in range(J):
        cwj = (FOURIER_C[j] * w_v).astype(np.float32)
        cvec[:64, 5 + j] = cwj
        cvec[64:, 5 + j] = cwj
    return wqk, cvec


def kernel(queries, keys, values, W_q, W_k, w_v, _trace=False, _trace_kwargs=None):
    from concourse.bass_utils import run_bass_kernel_spmd

    nc = _get_nc()
    wqk, cvec = _make_consts(
        np.asarray(W_q), np.asarray(W_k), np.asarray(w_v))
    queries = np.ascontiguousarray(queries, np.float32)
    keys = np.ascontiguousarray(keys, np.float32)
    values = np.ascontiguousarray(values, np.float32)

    in_maps = []
    for c in range(NCORES):
        b, qh = c // 2, c % 2
        in_maps.append({
            "q_sh": np.ascontiguousarray(queries[b, qh * NQH:(qh + 1) * NQH, :]),
            "k_sh": keys[b],
            "v_sh": values[b],
            "wqk": wqk, "cvec": cvec,
        })

    kwargs = {}
    if _trace:
        kwargs["trace"] = True
        kwargs.update(_trace_kwargs or {})
    res = run_bass_kernel_spmd(nc, in_maps, core_ids=list(range(NCORES)), **kwargs)

    out = np.empty((BS, NQ, VD), np.float32)
    for c in range(NCORES):
        b, qh = c // 2, c % 2
        out[b, qh * NQH:(qh + 1) * NQH, :] = res.results[c]["out"]
    if _trace:
        return out, res
    return out
